# revision 26
# baseline (speedup 1.0000x reference)
"""COLoRA linear kernel for 8 Trainium2 NeuronCores.

Reference computation (per batch element b with task t = task_ids[b]):

    out[b] = x[b] @ W.T + bias
           + cw      * 2 * (x[b] @ shared_A.T)    @ shared_B.T
           + (1-cw)  * 2 * (x[b] @ expert_A[t].T) @ expert_B[t].T
    cw = sigmoid(collab_w)

The rank-8 adapters fold exactly into the dense weight (associativity):

    W_eff[b] = W + cw*2*(shared_B @ shared_A) + (1-cw)*2*(expert_B[t] @ expert_A[t])
    out[b]   = x[b] @ W_eff[b].T + bias

so the device kernel is a single GEMM per core; core c handles batch
element c (B == n_cores == 8); task_ids routing happens on the host.

Design, from HW traces of nine variants (best measured: 129,837 ns vs
the 139,523 ns baseline; PE floor is 512 matmuls x 216 ns = 110.6 us
plus ~12 us of fixed NEFF preamble/epilogue/write-receipt):
  * bf16 matmuls pace at 216 ns/MM (the N/2.4GHz ideal), float32r at
    227; mixed 32/16-bit operands are rejected by the compiler.  So
    everything is bf16: loads drop to 10.5 MB (fp32 loads measurably
    starved the ramp at the ~345 GB/s delivered HBM rate), stores
    8.4 MB (host upcasts the output; rel err ~3.6e-3 vs the 2e-2 gate).
  * Whole x + W are SBUF-resident (80 KiB/partition), loads issued
    up-front in exact PE consumption order; after the ramp the PE
    stream runs gap-free at 216 ns/MM.
  * Ramp: phase A computes the h0 output half of rows 0..1023 k-outer
    across all 8 psum banks, so each arriving (x[t0,k], x[t1,k],
    Wh0[k]) chunk triple feeds 8 matmuls and the PE never idles through
    the HAM clock ramp (PE starts at 1.2 GHz; ~3.4 us of sustained
    activity un-throttles it to 2.4 GHz, and >3.4 us of idle
    re-throttles).  Phase B (h1 of the same rows) runs u-outer from
    resident data while phase A's staggered evacuations free banks.
  * Steady region (rows 1024+): per-128-row tiles as two sequential
    8-matmul accumulation groups (h0 then h1), bias-add evacuation on
    the vector engine, one 128 KiB store per half.
  * Output half-tiles come from a 16-deep pool: a 4-deep pool once let
    stores (queued FIFO behind a weight load on the same ring) block
    tile reuse -> psum starvation -> 19.7 us PE stall + re-throttle.
  * Warmup matmuls are kept live via a zero-graft into the bias tile
    (plain warmups into a never-read psum get dead-code-eliminated by
    the compiler; the PE then sat cold at 1.2 GHz until 12.7 us).
  * Store rings: scalar while the sync ring still streams x, then
    alternate; the final tile's halves store on both rings as each
    bias-add lands to shorten the exit drain.
  * Note: the chip sporadically enters the P0 power state (PE at
    2.0 GHz instead of 2.4; matmul spacing reads 259 ns) - chip-level
    and run-variable, observed to cluster in time.  Nothing in the
    kernel controls it; compare runs by spacing, not raw exec time.
"""

import os

import numpy as np

import concourse.bass as bass
import concourse.tile as tile
from concourse import bacc, mybir
from concourse.bass_utils import run_bass_kernel_spmd

try:  # tracing (BASS_TRACE) needs the axon NTFF hook; scrub if unavailable
    from antenv.axon_hooks import get_axon_ntff_profile_hook  # noqa: F401
except ImportError:
    os.environ.pop("BASS_TRACE", None)

N_CORES = 8
S = 4096        # rows per core (sequence length; one batch element per core)
D_IN = 1024
D_OUT = 1024
KC = D_IN // 128   # contraction chunks of 128
S_MACRO = 512      # s rows per macro (x DMA granularity)
N_HALF = 512       # psum free dim (one bank)
SCALING = 2.0      # lora alpha/r = 16/8

MM_DT = mybir.dt.bfloat16
OUT_DT = mybir.dt.bfloat16
N_WARM = 5
NG = 8             # ramp groups: rows 0..1023 (macros t=0,1)

_PROGRAM = None
LAST_RESULTS = None  # test harness introspection (exec_time_ns when traced)


def _build_program():
    f32 = mybir.dt.float32
    nc = bacc.Bacc("TRN2", debug=False, num_devices=N_CORES)

    xt_d = nc.dram_tensor("xt", [D_IN, S], MM_DT, kind="ExternalInput").ap()
    wt_d = nc.dram_tensor("wt", [D_IN, D_OUT], MM_DT, kind="ExternalInput").ap()
    bb_d = nc.dram_tensor("bb", [128, D_OUT], OUT_DT, kind="ExternalInput").ap()
    out_d = nc.dram_tensor("out", [S, D_OUT], OUT_DT, kind="ExternalOutput").ap()

    # contraction dim on partitions, chunked by 128
    xt_v = xt_d.rearrange("(k p) s -> p k s", p=128)      # [128, KC, S]
    wt_v = wt_d.rearrange("(k p) o -> p k o", p=128)      # [128, KC, D_OUT]
    out_v = out_d.rearrange("(n p) o -> n p o", p=128)    # [32, 128, D_OUT]

    NT = S // S_MACRO
    NH = D_OUT // N_HALF
    N_TILES = S // 128

    with tile.TileContext(nc) as tc:
        with (
            tc.tile_pool(name="const", bufs=1) as cpool,
            tc.tile_pool(name="outp", bufs=16) as opool,
            tc.tile_pool(name="psum", bufs=8, space="PSUM") as ppool,
        ):
            # PE HAM warmup: one live accumulation group with no DMA deps.
            # Its (zero) result is grafted into the bias tile below so dead
            # code elimination cannot drop it.
            warm_w = cpool.tile([128, 128], MM_DT)
            warm_x = cpool.tile([128, N_HALF], MM_DT)
            nc.gpsimd.memset(warm_w[:], 0.0)
            nc.gpsimd.memset(warm_x[:], 0.0)
            warm_ps = ppool.tile([128, N_HALF], f32, tag="ps")
            for i in range(N_WARM):
                nc.tensor.matmul(
                    warm_ps[:], warm_w[:], warm_x[:],
                    start=(i == 0), stop=(i == N_WARM - 1),
                )
            # evacuate immediately (no DMA dep) so the warm psum bank frees
            # before the ramp needs all 8 banks
            warm_sb = cpool.tile([128, N_HALF], f32)
            nc.vector.tensor_scalar_add(warm_sb[:], warm_ps[:], 0.0)

            # scalar ring: W h0 chunks (the ramp's first need), bias, W h1.
            # The k0 chunk that gates the very first matmul is split in two
            # so the PE can start on its first 64 KiB.
            wtile = cpool.tile([128, KC, D_OUT], MM_DT)
            nc.scalar.dma_start(wtile[:, 0, : N_HALF // 2], wt_v[:, 0, : N_HALF // 2])
            nc.scalar.dma_start(
                wtile[:, 0, N_HALF // 2 : N_HALF], wt_v[:, 0, N_HALF // 2 : N_HALF]
            )
            for k in range(1, KC):
                nc.scalar.dma_start(wtile[:, k, :N_HALF], wt_v[:, k, :N_HALF])
            btile = cpool.tile([128, D_OUT], OUT_DT)
            nc.scalar.dma_start(btile[:], bb_d[:])
            for k in range(KC):
                nc.scalar.dma_start(wtile[:, k, N_HALF:], wt_v[:, k, N_HALF:])

            # sync ring: x in ramp consumption order (per (k,t) for the two
            # ramp macros), then one DMA per remaining macro
            xfull = cpool.tile([128, KC, S], MM_DT)
            # the very first matmul only reads x[k0, rows 0..127]; load that
            # 32 KiB on its own so the PE starts ~1us earlier
            nc.sync.dma_start(xfull[:, 0, :128], xt_v[:, 0, :128])
            nc.sync.dma_start(xfull[:, 0, 128:S_MACRO], xt_v[:, 0, 128:S_MACRO])
            for k in range(KC):
                for t in range(2):
                    if k == 0 and t == 0:
                        continue
                    s_sl = slice(t * S_MACRO, (t + 1) * S_MACRO)
                    nc.sync.dma_start(xfull[:, k, s_sl], xt_v[:, k, s_sl])
            for t in range(2, NT):
                s_sl = slice(t * S_MACRO, (t + 1) * S_MACRO)
                nc.sync.dma_start(xfull[:, :, s_sl], xt_v[:, :, s_sl])

            # graft the (zero) warmup result into the bias tile: keeps the
            # warmup live, costs one DVE op before the first evacuation
            nc.vector.tensor_add(btile[:, :N_HALF], btile[:, :N_HALF], warm_sb[:])

            # phase A ramp: h0 half of rows 0..1023, k outermost across all
            # 8 psum banks
            psA = []
            for g in range(NG):
                ps = ppool.tile([128, N_HALF], f32, tag="ps")
                psA.append(ps)
            for k in range(KC):
                for g in range(NG):
                    if k == 0 and g == 0:
                        # first real matmul in two N=256 pieces: piece 1
                        # only needs the first 64 KiB of W.  start=True
                        # clears the bank and marks cols 0..255 written;
                        # piece 2 (start=False) overwrites its unwritten
                        # cols, later k accumulate everywhere - exact.
                        nc.tensor.matmul(
                            psA[0][:, : N_HALF // 2],
                            xfull[:, 0, :128],
                            wtile[:, 0, : N_HALF // 2],
                            start=True, stop=False,
                        )
                        nc.tensor.matmul(
                            psA[0][:, N_HALF // 2 :],
                            xfull[:, 0, :128],
                            wtile[:, 0, N_HALF // 2 : N_HALF],
                            start=False, stop=False,
                        )
                        continue
                    nc.tensor.matmul(
                        psA[g][:],
                        xfull[:, k, g * 128 : (g + 1) * 128],
                        wtile[:, k, :N_HALF],
                        start=(k == 0),
                        stop=(k == KC - 1),
                    )
            for g in range(NG):
                ot = opool.tile([128, N_HALF], OUT_DT)
                nc.vector.tensor_add(ot[:], psA[g][:], btile[:, :N_HALF])
                nc.scalar.dma_start(out_v[g][:, :N_HALF], ot[:])

            # phase B: h1 half of rows 0..1023, u-outer; then the steady
            # region rows 1024+ as sequential per-half accumulation groups.
            # Uniform inner shape: 8 matmuls k-inner into one bank, bias-add
            # evacuation, 128 KiB store.
            for n in range(N_TILES):
                halves = (1,) if n < NG else (0, 1)
                for h in halves:
                    if n == N_TILES - 1 and h == 1:
                        # very last half: two sequential N=256 accumulation
                        # groups in separate banks, so piece 1's bias-add,
                        # store and HBM write receipt all overlap piece 2's
                        # matmuls - only a 64 KiB add+store+receipt remains
                        # after the final matmul
                        for q in range(2):
                            osl = slice(N_HALF + q * 256, N_HALF + (q + 1) * 256)
                            ps = ppool.tile([128, N_HALF // 2], f32, tag="ps")
                            ot = opool.tile([128, N_HALF // 2], OUT_DT)
                            for k in range(KC):
                                nc.tensor.matmul(
                                    ps[:],
                                    xfull[:, k, n * 128 : (n + 1) * 128],
                                    wtile[:, k, osl],
                                    start=(k == 0),
                                    stop=(k == KC - 1),
                                )
                            nc.vector.tensor_add(ot[:], ps[:], btile[:, osl])
                            eng = nc.scalar if q == 0 else nc.sync
                            eng.dma_start(out_v[n][:, osl], ot[:])
                        continue
                    ps = ppool.tile([128, N_HALF], f32, tag="ps")
                    ot = opool.tile([128, N_HALF], OUT_DT)
                    for k in range(KC):
                        nc.tensor.matmul(
                            ps[:],
                            xfull[:, k, n * 128 : (n + 1) * 128],
                            wtile[:, k, h * N_HALF : (h + 1) * N_HALF],
                            start=(k == 0),
                            stop=(k == KC - 1),
                        )
                    nc.vector.tensor_add(
                        ot[:], ps[:], btile[:, h * N_HALF : (h + 1) * N_HALF]
                    )
                    if n == N_TILES - 1:
                        # final tile h0: overlaps h1's matmuls
                        eng = nc.scalar
                    elif n < 16:
                        # sync ring still owns the x-load stream
                        eng = nc.scalar
                    else:
                        eng = nc.scalar if (2 * n + h) % 2 == 0 else nc.sync
                    eng.dma_start(
                        out_v[n][:, h * N_HALF : (h + 1) * N_HALF], ot[:]
                    )

    nc.compile()
    return nc


def _get_program():
    global _PROGRAM
    if _PROGRAM is None:
        _PROGRAM = _build_program()
    return _PROGRAM


def kernel(x, task_ids, W, b, shared_A, shared_B, expert_A, expert_B, collab_w):
    global LAST_RESULTS
    x = np.asarray(x, dtype=np.float32)
    task_ids = np.asarray(task_ids)
    W = np.asarray(W, dtype=np.float32)
    b = np.asarray(b, dtype=np.float32)
    B = x.shape[0]
    assert B == N_CORES and x.shape[1:] == (S, D_IN)

    cw = np.float32(1.0 / (1.0 + np.exp(-np.float64(collab_w))))
    w_shared = (
        W
        + np.float32(cw * SCALING)
        * (np.asarray(shared_B, np.float32) @ np.asarray(shared_A, np.float32))
    ).astype(np.float32)
    ce = np.float32((1.0 - cw) * SCALING)

    np_in = mybir.dt.np(MM_DT)
    np_out = mybir.dt.np(OUT_DT)
    bb = np.ascontiguousarray(np.broadcast_to(b, (128, D_OUT))).astype(np_out)
    in_maps = []
    for bi in range(B):
        t = int(task_ids[bi])
        w_eff = w_shared + ce * (
            np.asarray(expert_B[t], np.float32) @ np.asarray(expert_A[t], np.float32)
        )
        in_maps.append(
            {
                "xt": np.ascontiguousarray(x[bi].T).astype(np_in),
                "wt": np.ascontiguousarray(w_eff.T).astype(np_in),
                "bb": bb,
            }
        )

    nc = _get_program()
    LAST_RESULTS = run_bass_kernel_spmd(nc, in_maps, list(range(N_CORES)))
    out = np.stack(
        [np.asarray(LAST_RESULTS.results[c]["out"]) for c in range(N_CORES)],
        axis=0,
    )
    return np.ascontiguousarray(out.astype(np.float32))


# revision 27
# speedup vs baseline: 1.0480x; 1.0480x over previous
"""COLoRA linear kernel for 8 Trainium2 NeuronCores.

Reference computation (per batch element b with task t = task_ids[b]):

    out[b] = x[b] @ W.T + bias
           + cw      * 2 * (x[b] @ shared_A.T)    @ shared_B.T
           + (1-cw)  * 2 * (x[b] @ expert_A[t].T) @ expert_B[t].T
    cw = sigmoid(collab_w)

The rank-8 adapters fold exactly into the dense weight (associativity):

    W_eff[b] = W + cw*2*(shared_B @ shared_A) + (1-cw)*2*(expert_B[t] @ expert_A[t])
    out[b]   = x[b] @ W_eff[b].T + bias

so the device kernel is a single GEMM per core; core c handles batch
element c (B == n_cores == 8); task_ids routing happens on the host.

Design, from HW traces of nine variants (best measured: 129,837 ns vs
the 139,523 ns baseline; PE floor is 512 matmuls x 216 ns = 110.6 us
plus ~12 us of fixed NEFF preamble/epilogue/write-receipt):
  * bf16 matmuls pace at 216 ns/MM (the N/2.4GHz ideal), float32r at
    227; mixed 32/16-bit operands are rejected by the compiler.  So
    everything is bf16: loads drop to 10.5 MB (fp32 loads measurably
    starved the ramp at the ~345 GB/s delivered HBM rate), stores
    8.4 MB (host upcasts the output; rel err ~3.6e-3 vs the 2e-2 gate).
  * Whole x + W are SBUF-resident (80 KiB/partition), loads issued
    up-front in exact PE consumption order; after the ramp the PE
    stream runs gap-free at 216 ns/MM.
  * Ramp: phase A computes the h0 output half of rows 0..1023 k-outer
    across all 8 psum banks, so each arriving (x[t0,k], x[t1,k],
    Wh0[k]) chunk triple feeds 8 matmuls and the PE never idles through
    the HAM clock ramp (PE starts at 1.2 GHz; ~3.4 us of sustained
    activity un-throttles it to 2.4 GHz, and >3.4 us of idle
    re-throttles).  Phase B (h1 of the same rows) runs u-outer from
    resident data while phase A's staggered evacuations free banks.
  * Steady region (rows 1024+): per-128-row tiles as two sequential
    8-matmul accumulation groups (h0 then h1), bias-add evacuation on
    the vector engine, one 128 KiB store per half.
  * Output half-tiles come from a 16-deep pool: a 4-deep pool once let
    stores (queued FIFO behind a weight load on the same ring) block
    tile reuse -> psum starvation -> 19.7 us PE stall + re-throttle.
  * Warmup matmuls are kept live via a zero-graft into the bias tile
    (plain warmups into a never-read psum get dead-code-eliminated by
    the compiler; the PE then sat cold at 1.2 GHz until 12.7 us).
  * Store rings: scalar while the sync ring still streams x, then
    alternate; the final tile's halves store on both rings as each
    bias-add lands to shorten the exit drain.
  * Note: the chip sporadically enters the P0 power state (PE at
    2.0 GHz instead of 2.4; matmul spacing reads 259 ns) - chip-level
    and run-variable, observed to cluster in time.  Nothing in the
    kernel controls it; compare runs by spacing, not raw exec time.
"""

import os

import numpy as np

import concourse.bass as bass
import concourse.tile as tile
from concourse import bacc, mybir
from concourse.bass_utils import run_bass_kernel_spmd

try:  # tracing (BASS_TRACE) needs the axon NTFF hook; scrub if unavailable
    from antenv.axon_hooks import get_axon_ntff_profile_hook  # noqa: F401
except ImportError:
    os.environ.pop("BASS_TRACE", None)

N_CORES = 8
S = 4096        # rows per core (sequence length; one batch element per core)
D_IN = 1024
D_OUT = 1024
KC = D_IN // 128   # contraction chunks of 128
S_MACRO = 512      # s rows per macro (x DMA granularity)
N_HALF = 512       # psum free dim (one bank)
SCALING = 2.0      # lora alpha/r = 16/8

MM_DT = mybir.dt.bfloat16
OUT_DT = mybir.dt.bfloat16
N_WARM = 5
NG = 8             # ramp groups: rows 0..1023 (macros t=0,1)

_PROGRAM = None
LAST_RESULTS = None  # test harness introspection (exec_time_ns when traced)


def _build_program():
    f32 = mybir.dt.float32
    nc = bacc.Bacc("TRN2", debug=False, num_devices=N_CORES)

    xt_d = nc.dram_tensor("xt", [D_IN, S], MM_DT, kind="ExternalInput").ap()
    wt_d = nc.dram_tensor("wt", [D_IN, D_OUT], MM_DT, kind="ExternalInput").ap()
    bb_d = nc.dram_tensor("bb", [128, D_OUT], OUT_DT, kind="ExternalInput").ap()
    out_d = nc.dram_tensor("out", [S, D_OUT], OUT_DT, kind="ExternalOutput").ap()

    # contraction dim on partitions, chunked by 128
    xt_v = xt_d.rearrange("(k p) s -> p k s", p=128)      # [128, KC, S]
    wt_v = wt_d.rearrange("(k p) o -> p k o", p=128)      # [128, KC, D_OUT]
    out_v = out_d.rearrange("(n p) o -> n p o", p=128)    # [32, 128, D_OUT]

    NT = S // S_MACRO
    NH = D_OUT // N_HALF
    N_TILES = S // 128

    with tile.TileContext(nc) as tc:
        with (
            tc.tile_pool(name="const", bufs=1) as cpool,
            tc.tile_pool(name="outp", bufs=16) as opool,
            tc.tile_pool(name="psum", bufs=8, space="PSUM") as ppool,
        ):
            # PE HAM warmup: one live accumulation group with no DMA deps.
            # Its (zero) result is grafted into the bias tile below so dead
            # code elimination cannot drop it.
            warm_w = cpool.tile([128, 128], MM_DT)
            warm_x = cpool.tile([128, N_HALF], MM_DT)
            nc.gpsimd.memset(warm_w[:], 0.0)
            nc.gpsimd.memset(warm_x[:], 0.0)
            warm_ps = ppool.tile([128, N_HALF], f32, tag="ps")
            for i in range(N_WARM):
                nc.tensor.matmul(
                    warm_ps[:], warm_w[:], warm_x[:],
                    start=(i == 0), stop=(i == N_WARM - 1),
                )
            # evacuate immediately (no DMA dep) so the warm psum bank frees
            # before the ramp needs all 8 banks
            warm_sb = cpool.tile([128, N_HALF], f32)
            nc.vector.tensor_scalar_add(warm_sb[:], warm_ps[:], 0.0)

            # scalar ring: W h0 chunks (the ramp's first need), bias, W h1.
            # The k0 chunk that gates the very first matmul is split in two
            # so the PE can start on its first 64 KiB.
            wtile = cpool.tile([128, KC, D_OUT], MM_DT)
            nc.scalar.dma_start(wtile[:, 0, : N_HALF // 2], wt_v[:, 0, : N_HALF // 2])
            nc.scalar.dma_start(
                wtile[:, 0, N_HALF // 2 : N_HALF], wt_v[:, 0, N_HALF // 2 : N_HALF]
            )
            for k in range(1, KC):
                nc.scalar.dma_start(wtile[:, k, :N_HALF], wt_v[:, k, :N_HALF])
            btile = cpool.tile([128, D_OUT], OUT_DT)
            nc.scalar.dma_start(btile[:], bb_d[:])
            for k in range(KC):
                nc.scalar.dma_start(wtile[:, k, N_HALF:], wt_v[:, k, N_HALF:])

            # sync ring: x in ramp consumption order (per (k,t) for the two
            # ramp macros), then one DMA per remaining macro
            xfull = cpool.tile([128, KC, S], MM_DT)
            # the very first matmul only reads x[k0, rows 0..127]; load that
            # 32 KiB on its own so the PE starts ~1us earlier
            nc.sync.dma_start(xfull[:, 0, :128], xt_v[:, 0, :128])
            nc.sync.dma_start(xfull[:, 0, 128:S_MACRO], xt_v[:, 0, 128:S_MACRO])
            for k in range(KC):
                for t in range(2):
                    if k == 0 and t == 0:
                        continue
                    s_sl = slice(t * S_MACRO, (t + 1) * S_MACRO)
                    nc.sync.dma_start(xfull[:, k, s_sl], xt_v[:, k, s_sl])
            for t in range(2, NT):
                s_sl = slice(t * S_MACRO, (t + 1) * S_MACRO)
                nc.sync.dma_start(xfull[:, :, s_sl], xt_v[:, :, s_sl])

            # graft the (zero) warmup result into the bias tile: keeps the
            # warmup live, costs one DVE op before the first evacuation
            nc.vector.tensor_add(btile[:, :N_HALF], btile[:, :N_HALF], warm_sb[:])

            # phase A ramp: h0 half of rows 0..1023, k outermost across all
            # 8 psum banks
            psA = []
            for g in range(NG):
                ps = ppool.tile([128, N_HALF], f32, tag="ps")
                psA.append(ps)
            for k in range(KC):
                for g in range(NG):
                    if k == 0 and g == 0:
                        # first real matmul in two N=256 pieces: piece 1
                        # only needs the first 64 KiB of W.  start=True
                        # clears the bank and marks cols 0..255 written;
                        # piece 2 (start=False) overwrites its unwritten
                        # cols, later k accumulate everywhere - exact.
                        nc.tensor.matmul(
                            psA[0][:, : N_HALF // 2],
                            xfull[:, 0, :128],
                            wtile[:, 0, : N_HALF // 2],
                            start=True, stop=False,
                        )
                        nc.tensor.matmul(
                            psA[0][:, N_HALF // 2 :],
                            xfull[:, 0, :128],
                            wtile[:, 0, N_HALF // 2 : N_HALF],
                            start=False, stop=False,
                        )
                        continue
                    nc.tensor.matmul(
                        psA[g][:],
                        xfull[:, k, g * 128 : (g + 1) * 128],
                        wtile[:, k, :N_HALF],
                        start=(k == 0),
                        stop=(k == KC - 1),
                    )
            for g in range(NG):
                ot = opool.tile([128, N_HALF], OUT_DT)
                nc.vector.tensor_add(ot[:], psA[g][:], btile[:, :N_HALF])
                nc.scalar.dma_start(out_v[g][:, :N_HALF], ot[:])

            # phase B: h1 half of rows 0..1023, u-outer; then the steady
            # region rows 1024+ as sequential per-half accumulation groups.
            # Uniform inner shape: 8 matmuls k-inner into one bank, bias-add
            # evacuation, 128 KiB store.
            for n in range(N_TILES):
                halves = (1,) if n < NG else (0, 1)
                for h in halves:
                    ps = ppool.tile([128, N_HALF], f32, tag="ps")
                    ot = opool.tile([128, N_HALF], OUT_DT)
                    for k in range(KC):
                        nc.tensor.matmul(
                            ps[:],
                            xfull[:, k, n * 128 : (n + 1) * 128],
                            wtile[:, k, h * N_HALF : (h + 1) * N_HALF],
                            start=(k == 0),
                            stop=(k == KC - 1),
                        )
                    if n == N_TILES - 1 and h == 1:
                        # very last store: two 256-col pieces on both rings
                        # so the final transfer + HBM write receipt (the
                        # exit drain's critical path) is half as deep
                        for q in range(2):
                            qsl = slice(q * 256, (q + 1) * 256)
                            osl = slice(N_HALF + q * 256, N_HALF + (q + 1) * 256)
                            nc.vector.tensor_add(ot[:, qsl], ps[:, qsl], btile[:, osl])
                            eng = nc.scalar if q == 0 else nc.sync
                            eng.dma_start(out_v[n][:, osl], ot[:, qsl])
                        continue
                    nc.vector.tensor_add(
                        ot[:], ps[:], btile[:, h * N_HALF : (h + 1) * N_HALF]
                    )
                    if n == N_TILES - 1:
                        # final tile h0: overlaps h1's matmuls
                        eng = nc.scalar
                    elif n < 16:
                        # sync ring still owns the x-load stream
                        eng = nc.scalar
                    else:
                        eng = nc.scalar if (2 * n + h) % 2 == 0 else nc.sync
                    eng.dma_start(
                        out_v[n][:, h * N_HALF : (h + 1) * N_HALF], ot[:]
                    )

    nc.compile()
    return nc


def _get_program():
    global _PROGRAM
    if _PROGRAM is None:
        _PROGRAM = _build_program()
    return _PROGRAM


def kernel(x, task_ids, W, b, shared_A, shared_B, expert_A, expert_B, collab_w):
    global LAST_RESULTS
    x = np.asarray(x, dtype=np.float32)
    task_ids = np.asarray(task_ids)
    W = np.asarray(W, dtype=np.float32)
    b = np.asarray(b, dtype=np.float32)
    B = x.shape[0]
    assert B == N_CORES and x.shape[1:] == (S, D_IN)

    cw = np.float32(1.0 / (1.0 + np.exp(-np.float64(collab_w))))
    w_shared = (
        W
        + np.float32(cw * SCALING)
        * (np.asarray(shared_B, np.float32) @ np.asarray(shared_A, np.float32))
    ).astype(np.float32)
    ce = np.float32((1.0 - cw) * SCALING)

    np_in = mybir.dt.np(MM_DT)
    np_out = mybir.dt.np(OUT_DT)
    bb = np.ascontiguousarray(np.broadcast_to(b, (128, D_OUT))).astype(np_out)
    in_maps = []
    for bi in range(B):
        t = int(task_ids[bi])
        w_eff = w_shared + ce * (
            np.asarray(expert_B[t], np.float32) @ np.asarray(expert_A[t], np.float32)
        )
        in_maps.append(
            {
                "xt": np.ascontiguousarray(x[bi].T).astype(np_in),
                "wt": np.ascontiguousarray(w_eff.T).astype(np_in),
                "bb": bb,
            }
        )

    nc = _get_program()
    LAST_RESULTS = run_bass_kernel_spmd(nc, in_maps, list(range(N_CORES)))
    out = np.stack(
        [np.asarray(LAST_RESULTS.results[c]["out"]) for c in range(N_CORES)],
        axis=0,
    )
    return np.ascontiguousarray(out.astype(np.float32))
